# revision 18
# baseline (speedup 1.0000x reference)
"""SS2D (stubbed-scan Mamba2D block) Trainium2 kernel.

Math (the four directional scans of the reference collapse to identity):
    xz = x @ W_in^T ; v = causal_depthwise_conv4(xz[:, :512]) + b ; z = xz[:, 512:]
    out = (4 * v * sigmoid(v) * z * sigmoid(z)) @ W_out^T

Sharding: 8 cores, each takes half of one batch image = 8192 tokens
(+3 halo tokens for the causal conv window).

Per-core kernel:
  - x chunk passed as bf16 (8320, 256) = 3 halo + 8192 tokens + 125 zero pad
  - DMA-transpose x into SBUF as xT (2 x (128c, 8320t) bf16)
  - in_proj with conv folded in: v^T tile accumulates 8 matmuls
    (4 taps x 2 c-halves) with tap-shifted rhs column offsets; conv bias
    rides the sigmoid's per-partition bias and a scalar_tensor_tensor op.
  - sigmoid on ScalarE (PSUM -> SBUF bf16), silu products on VectorE
  - out_proj: y^T-stationary matmuls -> out tile (tokens x 256) in PSUM,
    copied to SBUF, contiguous DMA store. The 4x scale is folded into W_out.
"""
import sys

sys.path.insert(0, "/opt/trn_rl_repo")

from contextlib import ExitStack

import numpy as np
import ml_dtypes

import concourse.bass as bass
import concourse.tile as tile
from concourse import bacc, mybir
from concourse.alu_op_type import AluOpType
from concourse.bass_utils import run_bass_kernel_spmd

BF16 = ml_dtypes.bfloat16
N_CORES = 8
B, H, W_DIM, C = 4, 128, 128, 256
L = H * W_DIM
TOK = 8192          # tokens per core
ROWS = 8320         # 3 halo + TOK + 125 pad  (= 65 * 128, mult of 16 for xbar)
T = 512             # token block
NBLK = TOK // T
DI = 512            # d_inner
F32 = mybir.dt.float32
BF16_T = mybir.dt.bfloat16
SIG = mybir.ActivationFunctionType.Sigmoid

_NC_CACHE = None


def _emit_kernel(tc, out_ap, xcs, wx_src, wd_src, wz_t_src, wo_t_src, bias_src):
    nc = tc.nc
    with ExitStack() as ctx:
        consts = ctx.enter_context(tc.tile_pool(name="consts", bufs=1))
        xt_pool = ctx.enter_context(tc.tile_pool(name="xt", bufs=1))
        sb = ctx.enter_context(tc.tile_pool(name="sb", bufs=3))
        ysb = ctx.enter_context(tc.tile_pool(name="ysb", bufs=2))
        osb = ctx.enter_context(tc.tile_pool(name="osb", bufs=16))
        xbsb = ctx.enter_context(tc.tile_pool(name="xbsb", bufs=3))
        pr = ctx.enter_context(tc.tile_pool(name="pr", bufs=2, space="PSUM"))
        pv = ctx.enter_context(tc.tile_pool(name="pv", bufs=2, space="PSUM"))
        pz = ctx.enter_context(tc.tile_pool(name="pz", bufs=1, space="PSUM"))
        po = ctx.enter_context(tc.tile_pool(name="po", bufs=2, space="PSUM"))

        # --- transpose x into SBUF: xT[c][p=c_chan, col=chunk_row] ---
        # (issued before the weight loads so the first blocks' columns land
        # as early as possible; everything stays on the sync ring — mixing
        # DMA-transpose with copies across rings corrupts data on HW)
        xt = [
            xt_pool.tile([128, ROWS], BF16_T, tag=f"xt{c}", name=f"xt{c}")
            for c in range(2)
        ]

        def _load_weights(stage):
            if stage == 0:
                nc.sync.dma_start(out=wx_t, in_=wx_src)
                nc.sync.dma_start(out=wd_t, in_=wd_src)
            elif stage == 1:
                nc.sync.dma_start(out=wz_t, in_=wz_t_src)
            else:
                nc.sync.dma_start(out=wo_t, in_=wo_t_src)
                nc.sync.dma_start(out=bias_t, in_=bias_src)

        wx_t = consts.tile([128, 8, 128], BF16_T)    # (c, dt) x-branch in_proj
        wd_t = consts.tile([128, 16, 128], BF16_T)   # (k, dt) diag conv taps
        wz_t = consts.tile([128, 8, 128], BF16_T)    # (c, dt) z-branch in_proj
        wo_t = consts.tile([128, 4, 256], BF16_T)    # (dt) out_proj (x4 folded)
        bias_t = consts.tile([128, 4], F32)          # conv bias, column per d-tile
        SEG = 640
        for s in range(ROWS // SEG):
            for c in range(2):
                nc.sync.dma_start(
                    out=xt[c][:, s * SEG:(s + 1) * SEG],
                    in_=xcs[c][s * SEG:(s + 1) * SEG, :],
                    transpose=True,
                )
            if s <= 2:
                _load_weights(s)

        # --- xb halo tiles for block 0: in_proj of x tokens [-3, 0) ---
        ph = ctx.enter_context(tc.tile_pool(name="ph", bufs=1, space="PSUM"))
        xb_prev = []
        for dt in range(4):
            rh = ph.tile([128, 3], F32, tag="rhalo", name=f"rhalo{dt}")
            for c in range(2):
                nc.tensor.matmul(
                    rh,
                    lhsT=wx_t[:, c * 4 + dt, :],
                    rhs=xt[c][:, 0:3],
                    start=(c == 0), stop=(c == 1),
                )
            hb = consts.tile([128, 3], BF16_T, name=f"xbhalo{dt}")
            nc.scalar.copy(hb, rh)
            xb_prev.append(hb)

        # --- main loop over token blocks ---
        for j in range(NBLK):
            base = T * j
            ys = []
            for dt in range(4):
                bias_ap = bias_t[:, dt:dt + 1]
                # x-branch in_proj (xb covers x tokens [base, base+T));
                # conv taps k<3 additionally read the last 3-k columns of the
                # previous block's xb tile (or the j==0 halo tile).
                r = pr.tile([128, T], F32)
                for c in range(2):
                    nc.tensor.matmul(
                        r,
                        lhsT=wx_t[:, c * 4 + dt, :],
                        rhs=xt[c][:, base + 3:base + 3 + T],
                        start=(c == 0), stop=(c == 1),
                    )
                xb = xbsb.tile([128, T], BF16_T, tag=f"xb{dt}")
                nc.scalar.copy(xb, r)
                prev = xb_prev[dt]
                poff = prev.shape[-1] - 3   # halo tile is (128,3): poff=0
                v = pv.tile([128, T], F32)
                # tap k=3 first: full-width write with start=True clears the
                # bank; remaining taps are pure per-element accumulates.
                nc.tensor.matmul(
                    v,
                    lhsT=wd_t[:, 3 * 4 + dt, :],
                    rhs=xb[:, 0:T],
                    start=True, stop=False,
                    skip_group_check=True,
                )
                for k in range(3):
                    nc.tensor.matmul(
                        v[:, 0:3 - k],
                        lhsT=wd_t[:, k * 4 + dt, :],
                        rhs=prev[:, poff + k:poff + 3],
                        start=False, stop=False,
                        skip_group_check=True,
                    )
                    nc.tensor.matmul(
                        v[:, 3 - k:T],
                        lhsT=wd_t[:, k * 4 + dt, :],
                        rhs=xb[:, 0:T - 3 + k],
                        start=False, stop=(k == 2),
                        skip_group_check=True,
                    )
                xb_prev[dt] = xb
                sv = sb.tile([128, T], BF16_T, tag="sv")
                nc.scalar.activation(sv, v, SIG, bias=bias_ap)

                # z-branch
                z = pz.tile([128, T], F32)
                for c in range(2):
                    nc.tensor.matmul(
                        z,
                        lhsT=wz_t[:, c * 4 + dt, :],
                        rhs=xt[c][:, base + 3:base + 3 + T],
                        start=(c == 0),
                        stop=(c == 1),
                    )
                sz = sb.tile([128, T], BF16_T, tag="sz")
                nc.scalar.activation(sz, z, SIG)

                # silu products: u = (v+b)*sig(v+b), zt = z*sig(z), y = u*zt
                u = sb.tile([128, T], BF16_T, tag="u")
                nc.vector.scalar_tensor_tensor(
                    u, v, bias_ap, sv, AluOpType.add, AluOpType.mult)
                zt = sb.tile([128, T], BF16_T, tag="zt")
                nc.vector.tensor_mul(zt, z, sz)
                y = ysb.tile([128, T], BF16_T, tag=f"y{dt}")
                nc.vector.tensor_mul(y, u, zt)
                ys.append(y)

            # out_proj: y^T tiles are stationary, W_out^T moving
            for s in range(T // 128):
                o_ps = po.tile([128, 256], F32)
                for dt in range(4):
                    nc.tensor.matmul(
                        o_ps,
                        lhsT=ys[dt][:, s * 128:(s + 1) * 128],
                        rhs=wo_t[:, dt, :],
                        start=(dt == 0),
                        stop=(dt == 3),
                    )
                o_sb = osb.tile([128, 256], F32)
                if s % 2 == 0:
                    nc.scalar.copy(o_sb, o_ps)
                else:
                    nc.vector.tensor_copy(o_sb, o_ps)
                nc.sync.dma_start(
                    out=out_ap[base + s * 128:base + (s + 1) * 128, :], in_=o_sb
                )


def _build_nc():
    global _NC_CACHE
    if _NC_CACHE is not None:
        return _NC_CACHE
    nc = bacc.Bacc("TRN2", target_bir_lowering=False, debug=False,
                   num_devices=N_CORES)
    xc0 = nc.dram_tensor("xc0", [ROWS, 128], BF16_T, kind="ExternalInput").ap()
    xc1 = nc.dram_tensor("xc1", [ROWS, 128], BF16_T, kind="ExternalInput").ap()
    wx = nc.dram_tensor("wx", [128, 8, 128], BF16_T, kind="ExternalInput").ap()
    wd = nc.dram_tensor("wd", [128, 16, 128], BF16_T, kind="ExternalInput").ap()
    wz = nc.dram_tensor("wz", [128, 8, 128], BF16_T, kind="ExternalInput").ap()
    wo = nc.dram_tensor("wo", [128, 4, 256], BF16_T, kind="ExternalInput").ap()
    bias = nc.dram_tensor("bias", [128, 4], F32, kind="ExternalInput").ap()
    out = nc.dram_tensor("out", [TOK, C], F32, kind="ExternalOutput").ap()

    with tile.TileContext(nc) as tc:
        _emit_kernel(tc, out, (xc0, xc1), wx, wd, wz, wo, bias)
    nc.compile()
    _NC_CACHE = nc
    return nc


def _pack_weights(W_in, conv_w, conv_b, W_out):
    w = conv_w[:, 0, :]                      # (512, 4)
    Wx, Wz = W_in[:DI], W_in[DI:]            # (512, 256) each

    wx = np.empty((128, 8, 128), np.float32)
    for c in range(2):
        for dt in range(4):
            blk = Wx[dt * 128:(dt + 1) * 128, c * 128:(c + 1) * 128]
            wx[:, c * 4 + dt, :] = blk.T
    wd = np.zeros((128, 16, 128), np.float32)
    idx = np.arange(128)
    for k in range(4):
        for dt in range(4):
            wd[idx, k * 4 + dt, idx] = w[dt * 128:(dt + 1) * 128, k]
    wz = np.empty((128, 8, 128), np.float32)
    for c in range(2):
        for dt in range(4):
            blk = Wz[dt * 128:(dt + 1) * 128, c * 128:(c + 1) * 128]
            wz[:, c * 4 + dt, :] = blk.T
    wo = np.empty((128, 4, 256), np.float32)
    WoT4 = 4.0 * W_out.T                     # (512, 256)
    for dt in range(4):
        wo[:, dt, :] = WoT4[dt * 128:(dt + 1) * 128, :]
    bias = conv_b.reshape(4, 128).T.copy()   # (128, 4), column dt
    return (wx.astype(BF16), wd.astype(BF16), wz.astype(BF16), wo.astype(BF16),
            np.ascontiguousarray(bias, np.float32))


def prepare_in_maps(x, W_in, conv_w, conv_b, W_out):
    wx, wd, wz, wo, bias = _pack_weights(
        np.asarray(W_in, np.float32), np.asarray(conv_w, np.float32),
        np.asarray(conv_b, np.float32), np.asarray(W_out, np.float32))

    xf = np.asarray(x, np.float32).reshape(B, L, C)
    in_maps = []
    for core in range(N_CORES):
        b, h = divmod(core, 2)
        chunk = np.zeros((ROWS, C), np.float32)
        if h == 1:
            chunk[0:3] = xf[b, TOK - 3:TOK]
        chunk[3:3 + TOK] = xf[b, h * TOK:(h + 1) * TOK]
        cb = chunk.astype(BF16)
        in_maps.append({
            "xc0": np.ascontiguousarray(cb[:, 0:128]),
            "xc1": np.ascontiguousarray(cb[:, 128:256]),
            "wx": wx, "wd": wd, "wz": wz, "wo": wo, "bias": bias,
        })
    return in_maps


def assemble_output(results):
    full = np.empty((B, L, C), np.float32)
    for core in range(N_CORES):
        b, h = divmod(core, 2)
        full[b, h * TOK:(h + 1) * TOK] = results[core]["out"]
    return full.reshape(B, H, W_DIM, C)


def kernel(x, W_in, conv_w, conv_b, W_out):
    nc = _build_nc()
    in_maps = prepare_in_maps(x, W_in, conv_w, conv_b, W_out)
    res = run_bass_kernel_spmd(nc, in_maps, list(range(N_CORES)))
    return assemble_output(res.results)


# revision 19
# speedup vs baseline: 1.0637x; 1.0637x over previous
"""SS2D (stubbed-scan Mamba2D block) Trainium2 kernel.

Math (the four directional scans of the reference collapse to identity):
    xz = x @ W_in^T ; v = causal_depthwise_conv4(xz[:, :512]) + b ; z = xz[:, 512:]
    out = (4 * v * sigmoid(v) * z * sigmoid(z)) @ W_out^T

Sharding: 8 cores, each takes half of one batch image = 8192 tokens
(+3 halo tokens for the causal conv window).

Per-core kernel:
  - x chunk passed as bf16 (8320, 256) = 3 halo + 8192 tokens + 125 zero pad
  - DMA-transpose x into SBUF as xT (2 x (128c, 8320t) bf16)
  - in_proj with conv folded in: v^T tile accumulates 8 matmuls
    (4 taps x 2 c-halves) with tap-shifted rhs column offsets; conv bias
    rides the sigmoid's per-partition bias and a scalar_tensor_tensor op.
  - sigmoid on ScalarE (PSUM -> SBUF bf16), silu products on VectorE
  - out_proj: y^T-stationary matmuls -> out tile (tokens x 256) in PSUM,
    copied to SBUF, contiguous DMA store. The 4x scale is folded into W_out.
"""
import sys

sys.path.insert(0, "/opt/trn_rl_repo")

from contextlib import ExitStack

import numpy as np
import ml_dtypes

import concourse.bass as bass
import concourse.tile as tile
from concourse import bacc, mybir
from concourse.alu_op_type import AluOpType
from concourse.bass_utils import run_bass_kernel_spmd

BF16 = ml_dtypes.bfloat16
N_CORES = 8
B, H, W_DIM, C = 4, 128, 128, 256
L = H * W_DIM
TOK = 8192          # tokens per core
ROWS = 8320         # 3 halo + TOK + 125 pad  (= 65 * 128, mult of 16 for xbar)
T = 512             # token block
NBLK = TOK // T
DI = 512            # d_inner
F32 = mybir.dt.float32
BF16_T = mybir.dt.bfloat16
SIG = mybir.ActivationFunctionType.Sigmoid

_NC_CACHE = None


def _emit_kernel(tc, out_ap, xcs, wx_src, wd_src, wz_t_src, wo_t_src, bias_src):
    nc = tc.nc
    with ExitStack() as ctx:
        consts = ctx.enter_context(tc.tile_pool(name="consts", bufs=1))
        xt_pool = ctx.enter_context(tc.tile_pool(name="xt", bufs=1))
        sb = ctx.enter_context(tc.tile_pool(name="sb", bufs=3))
        ysb = ctx.enter_context(tc.tile_pool(name="ysb", bufs=2))
        osb = ctx.enter_context(tc.tile_pool(name="osb", bufs=16))
        xbsb = ctx.enter_context(tc.tile_pool(name="xbsb", bufs=3))
        pr = ctx.enter_context(tc.tile_pool(name="pr", bufs=2, space="PSUM"))
        pv = ctx.enter_context(tc.tile_pool(name="pv", bufs=2, space="PSUM"))
        pz = ctx.enter_context(tc.tile_pool(name="pz", bufs=1, space="PSUM"))
        po = ctx.enter_context(tc.tile_pool(name="po", bufs=2, space="PSUM"))

        # --- transpose x into SBUF: xT[c][p=c_chan, col=chunk_row] ---
        # (issued before the weight loads so the first blocks' columns land
        # as early as possible; everything stays on the sync ring — mixing
        # DMA-transpose with copies across rings corrupts data on HW)
        xt = [
            xt_pool.tile([128, ROWS], BF16_T, tag=f"xt{c}", name=f"xt{c}")
            for c in range(2)
        ]

        def _load_weights():
            nc.sync.dma_start(out=wx_t, in_=wx_src)
            nc.sync.dma_start(out=wd_t, in_=wd_src)
            nc.sync.dma_start(out=wz_t, in_=wz_t_src)
            nc.sync.dma_start(out=wo_t, in_=wo_t_src)
            nc.sync.dma_start(out=bias_t, in_=bias_src)

        wx_t = consts.tile([128, 8, 128], BF16_T)    # (c, dt) x-branch in_proj
        wd_t = consts.tile([128, 16, 128], BF16_T)   # (k, dt) diag conv taps
        wz_t = consts.tile([128, 8, 128], BF16_T)    # (c, dt) z-branch in_proj
        wo_t = consts.tile([128, 4, 256], BF16_T)    # (dt) out_proj (x4 folded)
        bias_t = consts.tile([128, 4], F32)          # conv bias, column per d-tile
        SEG = 640
        for s in range(ROWS // SEG):
            for c in range(2):
                nc.sync.dma_start(
                    out=xt[c][:, s * SEG:(s + 1) * SEG],
                    in_=xcs[c][s * SEG:(s + 1) * SEG, :],
                    transpose=True,
                )
            if s == 1:
                _load_weights()

        # --- xb halo tiles for block 0: in_proj of x tokens [-3, 0) ---
        ph = ctx.enter_context(tc.tile_pool(name="ph", bufs=1, space="PSUM"))
        xb_prev = []
        for dt in range(4):
            rh = ph.tile([128, 3], F32, tag="rhalo", name=f"rhalo{dt}")
            for c in range(2):
                nc.tensor.matmul(
                    rh,
                    lhsT=wx_t[:, c * 4 + dt, :],
                    rhs=xt[c][:, 0:3],
                    start=(c == 0), stop=(c == 1),
                )
            hb = consts.tile([128, 3], BF16_T, name=f"xbhalo{dt}")
            nc.scalar.copy(hb, rh)
            xb_prev.append(hb)

        # --- main loop over token blocks ---
        for j in range(NBLK):
            base = T * j
            ys = []
            for dt in range(4):
                bias_ap = bias_t[:, dt:dt + 1]
                # x-branch in_proj (xb covers x tokens [base, base+T));
                # conv taps k<3 additionally read the last 3-k columns of the
                # previous block's xb tile (or the j==0 halo tile).
                r = pr.tile([128, T], F32)
                for c in range(2):
                    nc.tensor.matmul(
                        r,
                        lhsT=wx_t[:, c * 4 + dt, :],
                        rhs=xt[c][:, base + 3:base + 3 + T],
                        start=(c == 0), stop=(c == 1),
                    )
                xb = xbsb.tile([128, T], BF16_T, tag=f"xb{dt}")
                nc.scalar.copy(xb, r)
                prev = xb_prev[dt]
                poff = prev.shape[-1] - 3   # halo tile is (128,3): poff=0
                v = pv.tile([128, T], F32)
                # tap k=3 first: full-width write with start=True clears the
                # bank; remaining taps are pure per-element accumulates.
                nc.tensor.matmul(
                    v,
                    lhsT=wd_t[:, 3 * 4 + dt, :],
                    rhs=xb[:, 0:T],
                    start=True, stop=False,
                    skip_group_check=True,
                )
                for k in range(3):
                    nc.tensor.matmul(
                        v[:, 0:3 - k],
                        lhsT=wd_t[:, k * 4 + dt, :],
                        rhs=prev[:, poff + k:poff + 3],
                        start=False, stop=False,
                        skip_group_check=True,
                    )
                    nc.tensor.matmul(
                        v[:, 3 - k:T],
                        lhsT=wd_t[:, k * 4 + dt, :],
                        rhs=xb[:, 0:T - 3 + k],
                        start=False, stop=(k == 2),
                        skip_group_check=True,
                    )
                xb_prev[dt] = xb
                sv = sb.tile([128, T], BF16_T, tag="sv")
                nc.scalar.activation(sv, v, SIG, bias=bias_ap)

                # z-branch
                z = pz.tile([128, T], F32)
                for c in range(2):
                    nc.tensor.matmul(
                        z,
                        lhsT=wz_t[:, c * 4 + dt, :],
                        rhs=xt[c][:, base + 3:base + 3 + T],
                        start=(c == 0),
                        stop=(c == 1),
                    )
                sz = sb.tile([128, T], BF16_T, tag="sz")
                nc.scalar.activation(sz, z, SIG)

                # silu products: u = (v+b)*sig(v+b), zt = z*sig(z), y = u*zt
                u = sb.tile([128, T], BF16_T, tag="u")
                nc.vector.scalar_tensor_tensor(
                    u, v, bias_ap, sv, AluOpType.add, AluOpType.mult)
                zt = sb.tile([128, T], BF16_T, tag="zt")
                nc.vector.tensor_mul(zt, z, sz)
                y = ysb.tile([128, T], BF16_T, tag=f"y{dt}")
                nc.vector.tensor_mul(y, u, zt)
                ys.append(y)

            # out_proj: y^T tiles are stationary, W_out^T moving
            for s in range(T // 128):
                o_ps = po.tile([128, 256], F32)
                for dt in range(4):
                    nc.tensor.matmul(
                        o_ps,
                        lhsT=ys[dt][:, s * 128:(s + 1) * 128],
                        rhs=wo_t[:, dt, :],
                        start=(dt == 0),
                        stop=(dt == 3),
                    )
                o_sb = osb.tile([128, 256], F32)
                if s % 2 == 0:
                    nc.scalar.copy(o_sb, o_ps)
                else:
                    nc.vector.tensor_copy(o_sb, o_ps)
                nc.sync.dma_start(
                    out=out_ap[base + s * 128:base + (s + 1) * 128, :], in_=o_sb
                )


def _build_nc():
    global _NC_CACHE
    if _NC_CACHE is not None:
        return _NC_CACHE
    nc = bacc.Bacc("TRN2", target_bir_lowering=False, debug=False,
                   num_devices=N_CORES)
    xc0 = nc.dram_tensor("xc0", [ROWS, 128], BF16_T, kind="ExternalInput").ap()
    xc1 = nc.dram_tensor("xc1", [ROWS, 128], BF16_T, kind="ExternalInput").ap()
    wx = nc.dram_tensor("wx", [128, 8, 128], BF16_T, kind="ExternalInput").ap()
    wd = nc.dram_tensor("wd", [128, 16, 128], BF16_T, kind="ExternalInput").ap()
    wz = nc.dram_tensor("wz", [128, 8, 128], BF16_T, kind="ExternalInput").ap()
    wo = nc.dram_tensor("wo", [128, 4, 256], BF16_T, kind="ExternalInput").ap()
    bias = nc.dram_tensor("bias", [128, 4], F32, kind="ExternalInput").ap()
    out = nc.dram_tensor("out", [TOK, C], F32, kind="ExternalOutput").ap()

    with tile.TileContext(nc) as tc:
        _emit_kernel(tc, out, (xc0, xc1), wx, wd, wz, wo, bias)
    nc.compile()
    _NC_CACHE = nc
    return nc


def _pack_weights(W_in, conv_w, conv_b, W_out):
    w = conv_w[:, 0, :]                      # (512, 4)
    Wx, Wz = W_in[:DI], W_in[DI:]            # (512, 256) each

    wx = np.empty((128, 8, 128), np.float32)
    for c in range(2):
        for dt in range(4):
            blk = Wx[dt * 128:(dt + 1) * 128, c * 128:(c + 1) * 128]
            wx[:, c * 4 + dt, :] = blk.T
    wd = np.zeros((128, 16, 128), np.float32)
    idx = np.arange(128)
    for k in range(4):
        for dt in range(4):
            wd[idx, k * 4 + dt, idx] = w[dt * 128:(dt + 1) * 128, k]
    wz = np.empty((128, 8, 128), np.float32)
    for c in range(2):
        for dt in range(4):
            blk = Wz[dt * 128:(dt + 1) * 128, c * 128:(c + 1) * 128]
            wz[:, c * 4 + dt, :] = blk.T
    wo = np.empty((128, 4, 256), np.float32)
    WoT4 = 4.0 * W_out.T                     # (512, 256)
    for dt in range(4):
        wo[:, dt, :] = WoT4[dt * 128:(dt + 1) * 128, :]
    bias = conv_b.reshape(4, 128).T.copy()   # (128, 4), column dt
    return (wx.astype(BF16), wd.astype(BF16), wz.astype(BF16), wo.astype(BF16),
            np.ascontiguousarray(bias, np.float32))


def prepare_in_maps(x, W_in, conv_w, conv_b, W_out):
    wx, wd, wz, wo, bias = _pack_weights(
        np.asarray(W_in, np.float32), np.asarray(conv_w, np.float32),
        np.asarray(conv_b, np.float32), np.asarray(W_out, np.float32))

    xf = np.asarray(x, np.float32).reshape(B, L, C)
    in_maps = []
    for core in range(N_CORES):
        b, h = divmod(core, 2)
        chunk = np.zeros((ROWS, C), np.float32)
        if h == 1:
            chunk[0:3] = xf[b, TOK - 3:TOK]
        chunk[3:3 + TOK] = xf[b, h * TOK:(h + 1) * TOK]
        cb = chunk.astype(BF16)
        in_maps.append({
            "xc0": np.ascontiguousarray(cb[:, 0:128]),
            "xc1": np.ascontiguousarray(cb[:, 128:256]),
            "wx": wx, "wd": wd, "wz": wz, "wo": wo, "bias": bias,
        })
    return in_maps


def assemble_output(results):
    full = np.empty((B, L, C), np.float32)
    for core in range(N_CORES):
        b, h = divmod(core, 2)
        full[b, h * TOK:(h + 1) * TOK] = results[core]["out"]
    return full.reshape(B, H, W_DIM, C)


def kernel(x, W_in, conv_w, conv_b, W_out):
    nc = _build_nc()
    in_maps = prepare_in_maps(x, W_in, conv_w, conv_b, W_out)
    res = run_bass_kernel_spmd(nc, in_maps, list(range(N_CORES)))
    return assemble_output(res.results)


# revision 20
# speedup vs baseline: 1.0794x; 1.0148x over previous
"""SS2D (stubbed-scan Mamba2D block) Trainium2 kernel.

Math (the four directional scans of the reference collapse to identity):
    xz = x @ W_in^T ; v = causal_depthwise_conv4(xz[:, :512]) + b ; z = xz[:, 512:]
    out = (4 * v * sigmoid(v) * z * sigmoid(z)) @ W_out^T

Sharding: 8 cores, each takes half of one batch image = 8192 tokens
(+3 halo tokens for the causal conv window).

Per-core kernel:
  - x chunk passed as bf16 (8320, 256) = 3 halo + 8192 tokens + 125 zero pad
  - DMA-transpose x into SBUF as xT (2 x (128c, 8320t) bf16)
  - in_proj with conv folded in: v^T tile accumulates 8 matmuls
    (4 taps x 2 c-halves) with tap-shifted rhs column offsets; conv bias
    rides the sigmoid's per-partition bias and a scalar_tensor_tensor op.
  - sigmoid on ScalarE (PSUM -> SBUF bf16), silu products on VectorE
  - out_proj: y^T-stationary matmuls -> out tile (tokens x 256) in PSUM,
    copied to SBUF, contiguous DMA store. The 4x scale is folded into W_out.
"""
import sys

sys.path.insert(0, "/opt/trn_rl_repo")

from contextlib import ExitStack

import numpy as np
import ml_dtypes

import concourse.bass as bass
import concourse.tile as tile
from concourse import bacc, mybir
from concourse.alu_op_type import AluOpType
from concourse.bass_utils import run_bass_kernel_spmd

BF16 = ml_dtypes.bfloat16
N_CORES = 8
B, H, W_DIM, C = 4, 128, 128, 256
L = H * W_DIM
TOK = 8192          # tokens per core
ROWS = 8320         # 3 halo + TOK + 125 pad  (= 65 * 128, mult of 16 for xbar)
T = 512             # token block
NBLK = TOK // T
DI = 512            # d_inner
F32 = mybir.dt.float32
BF16_T = mybir.dt.bfloat16
SIG = mybir.ActivationFunctionType.Sigmoid

_NC_CACHE = None


def _emit_kernel(tc, out_ap, xcs, wx_src, wd_src, wz_t_src, wo_t_src, bias_src):
    nc = tc.nc
    with ExitStack() as ctx:
        consts = ctx.enter_context(tc.tile_pool(name="consts", bufs=1))
        xt_pool = ctx.enter_context(tc.tile_pool(name="xt", bufs=1))
        sb = ctx.enter_context(tc.tile_pool(name="sb", bufs=3))
        ysb = ctx.enter_context(tc.tile_pool(name="ysb", bufs=2))
        osb = ctx.enter_context(tc.tile_pool(name="osb", bufs=16))
        xbsb = ctx.enter_context(tc.tile_pool(name="xbsb", bufs=3))
        pr = ctx.enter_context(tc.tile_pool(name="pr", bufs=2, space="PSUM"))
        pv = ctx.enter_context(tc.tile_pool(name="pv", bufs=2, space="PSUM"))
        pz = ctx.enter_context(tc.tile_pool(name="pz", bufs=1, space="PSUM"))
        po = ctx.enter_context(tc.tile_pool(name="po", bufs=2, space="PSUM"))

        # --- transpose x into SBUF: xT[c][p=c_chan, col=chunk_row] ---
        # (issued before the weight loads so the first blocks' columns land
        # as early as possible; everything stays on the sync ring — mixing
        # DMA-transpose with copies across rings corrupts data on HW)
        xt = [
            xt_pool.tile([128, ROWS], BF16_T, tag=f"xt{c}", name=f"xt{c}")
            for c in range(2)
        ]

        def _load_weights():
            nc.sync.dma_start(out=wx_t, in_=wx_src)
            nc.sync.dma_start(out=wd_t, in_=wd_src)
            nc.sync.dma_start(out=wz_t, in_=wz_t_src)
            nc.sync.dma_start(out=wo_t, in_=wo_t_src)
            nc.sync.dma_start(out=bias_t, in_=bias_src)

        wx_t = consts.tile([128, 8, 128], BF16_T)    # (c, dt) x-branch in_proj
        wd_t = consts.tile([128, 16, 128], BF16_T)   # (k, dt) diag conv taps
        wz_t = consts.tile([128, 8, 128], BF16_T)    # (c, dt) z-branch in_proj
        wo_t = consts.tile([128, 4, 256], BF16_T)    # (dt) out_proj (x4 folded)
        bias_t = consts.tile([128, 4], F32)          # conv bias, column per d-tile
        SEG = 640
        for s in range(ROWS // SEG):
            for c in range(2):
                nc.sync.dma_start(
                    out=xt[c][:, s * SEG:(s + 1) * SEG],
                    in_=xcs[c][s * SEG:(s + 1) * SEG, :],
                    transpose=True,
                )
            if s == 0:
                _load_weights()

        # --- xb halo tiles for block 0: in_proj of x tokens [-3, 0) ---
        ph = ctx.enter_context(tc.tile_pool(name="ph", bufs=1, space="PSUM"))
        xb_prev = []
        for dt in range(4):
            rh = ph.tile([128, 3], F32, tag="rhalo", name=f"rhalo{dt}")
            for c in range(2):
                nc.tensor.matmul(
                    rh,
                    lhsT=wx_t[:, c * 4 + dt, :],
                    rhs=xt[c][:, 0:3],
                    start=(c == 0), stop=(c == 1),
                )
            hb = consts.tile([128, 3], BF16_T, name=f"xbhalo{dt}")
            nc.scalar.copy(hb, rh)
            xb_prev.append(hb)

        # --- main loop over token blocks ---
        for j in range(NBLK):
            base = T * j
            ys = []
            for dt in range(4):
                bias_ap = bias_t[:, dt:dt + 1]
                # x-branch in_proj (xb covers x tokens [base, base+T));
                # conv taps k<3 additionally read the last 3-k columns of the
                # previous block's xb tile (or the j==0 halo tile).
                r = pr.tile([128, T], F32)
                for c in range(2):
                    nc.tensor.matmul(
                        r,
                        lhsT=wx_t[:, c * 4 + dt, :],
                        rhs=xt[c][:, base + 3:base + 3 + T],
                        start=(c == 0), stop=(c == 1),
                    )
                xb = xbsb.tile([128, T], BF16_T, tag=f"xb{dt}")
                nc.scalar.copy(xb, r)
                prev = xb_prev[dt]
                poff = prev.shape[-1] - 3   # halo tile is (128,3): poff=0
                v = pv.tile([128, T], F32)
                # tap k=3 first: full-width write with start=True clears the
                # bank; remaining taps are pure per-element accumulates.
                nc.tensor.matmul(
                    v,
                    lhsT=wd_t[:, 3 * 4 + dt, :],
                    rhs=xb[:, 0:T],
                    start=True, stop=False,
                    skip_group_check=True,
                )
                for k in range(3):
                    nc.tensor.matmul(
                        v[:, 0:3 - k],
                        lhsT=wd_t[:, k * 4 + dt, :],
                        rhs=prev[:, poff + k:poff + 3],
                        start=False, stop=False,
                        skip_group_check=True,
                    )
                    nc.tensor.matmul(
                        v[:, 3 - k:T],
                        lhsT=wd_t[:, k * 4 + dt, :],
                        rhs=xb[:, 0:T - 3 + k],
                        start=False, stop=(k == 2),
                        skip_group_check=True,
                    )
                xb_prev[dt] = xb
                sv = sb.tile([128, T], BF16_T, tag="sv")
                nc.scalar.activation(sv, v, SIG, bias=bias_ap)

                # z-branch
                z = pz.tile([128, T], F32)
                for c in range(2):
                    nc.tensor.matmul(
                        z,
                        lhsT=wz_t[:, c * 4 + dt, :],
                        rhs=xt[c][:, base + 3:base + 3 + T],
                        start=(c == 0),
                        stop=(c == 1),
                    )
                sz = sb.tile([128, T], BF16_T, tag="sz")
                nc.scalar.activation(sz, z, SIG)

                # silu products: u = (v+b)*sig(v+b), zt = z*sig(z), y = u*zt
                u = sb.tile([128, T], BF16_T, tag="u")
                nc.vector.scalar_tensor_tensor(
                    u, v, bias_ap, sv, AluOpType.add, AluOpType.mult)
                zt = sb.tile([128, T], BF16_T, tag="zt")
                nc.vector.tensor_mul(zt, z, sz)
                y = ysb.tile([128, T], BF16_T, tag=f"y{dt}")
                nc.vector.tensor_mul(y, u, zt)
                ys.append(y)

            # out_proj: y^T tiles are stationary, W_out^T moving
            for s in range(T // 128):
                o_ps = po.tile([128, 256], F32)
                for dt in range(4):
                    nc.tensor.matmul(
                        o_ps,
                        lhsT=ys[dt][:, s * 128:(s + 1) * 128],
                        rhs=wo_t[:, dt, :],
                        start=(dt == 0),
                        stop=(dt == 3),
                    )
                o_sb = osb.tile([128, 256], F32)
                if s % 2 == 0:
                    nc.scalar.copy(o_sb, o_ps)
                else:
                    nc.vector.tensor_copy(o_sb, o_ps)
                nc.sync.dma_start(
                    out=out_ap[base + s * 128:base + (s + 1) * 128, :], in_=o_sb
                )


def _build_nc():
    global _NC_CACHE
    if _NC_CACHE is not None:
        return _NC_CACHE
    nc = bacc.Bacc("TRN2", target_bir_lowering=False, debug=False,
                   num_devices=N_CORES)
    xc0 = nc.dram_tensor("xc0", [ROWS, 128], BF16_T, kind="ExternalInput").ap()
    xc1 = nc.dram_tensor("xc1", [ROWS, 128], BF16_T, kind="ExternalInput").ap()
    wx = nc.dram_tensor("wx", [128, 8, 128], BF16_T, kind="ExternalInput").ap()
    wd = nc.dram_tensor("wd", [128, 16, 128], BF16_T, kind="ExternalInput").ap()
    wz = nc.dram_tensor("wz", [128, 8, 128], BF16_T, kind="ExternalInput").ap()
    wo = nc.dram_tensor("wo", [128, 4, 256], BF16_T, kind="ExternalInput").ap()
    bias = nc.dram_tensor("bias", [128, 4], F32, kind="ExternalInput").ap()
    out = nc.dram_tensor("out", [TOK, C], F32, kind="ExternalOutput").ap()

    with tile.TileContext(nc) as tc:
        _emit_kernel(tc, out, (xc0, xc1), wx, wd, wz, wo, bias)
    nc.compile()
    _NC_CACHE = nc
    return nc


def _pack_weights(W_in, conv_w, conv_b, W_out):
    w = conv_w[:, 0, :]                      # (512, 4)
    Wx, Wz = W_in[:DI], W_in[DI:]            # (512, 256) each

    wx = np.empty((128, 8, 128), np.float32)
    for c in range(2):
        for dt in range(4):
            blk = Wx[dt * 128:(dt + 1) * 128, c * 128:(c + 1) * 128]
            wx[:, c * 4 + dt, :] = blk.T
    wd = np.zeros((128, 16, 128), np.float32)
    idx = np.arange(128)
    for k in range(4):
        for dt in range(4):
            wd[idx, k * 4 + dt, idx] = w[dt * 128:(dt + 1) * 128, k]
    wz = np.empty((128, 8, 128), np.float32)
    for c in range(2):
        for dt in range(4):
            blk = Wz[dt * 128:(dt + 1) * 128, c * 128:(c + 1) * 128]
            wz[:, c * 4 + dt, :] = blk.T
    wo = np.empty((128, 4, 256), np.float32)
    WoT4 = 4.0 * W_out.T                     # (512, 256)
    for dt in range(4):
        wo[:, dt, :] = WoT4[dt * 128:(dt + 1) * 128, :]
    bias = conv_b.reshape(4, 128).T.copy()   # (128, 4), column dt
    return (wx.astype(BF16), wd.astype(BF16), wz.astype(BF16), wo.astype(BF16),
            np.ascontiguousarray(bias, np.float32))


def prepare_in_maps(x, W_in, conv_w, conv_b, W_out):
    wx, wd, wz, wo, bias = _pack_weights(
        np.asarray(W_in, np.float32), np.asarray(conv_w, np.float32),
        np.asarray(conv_b, np.float32), np.asarray(W_out, np.float32))

    xf = np.asarray(x, np.float32).reshape(B, L, C)
    in_maps = []
    for core in range(N_CORES):
        b, h = divmod(core, 2)
        chunk = np.zeros((ROWS, C), np.float32)
        if h == 1:
            chunk[0:3] = xf[b, TOK - 3:TOK]
        chunk[3:3 + TOK] = xf[b, h * TOK:(h + 1) * TOK]
        cb = chunk.astype(BF16)
        in_maps.append({
            "xc0": np.ascontiguousarray(cb[:, 0:128]),
            "xc1": np.ascontiguousarray(cb[:, 128:256]),
            "wx": wx, "wd": wd, "wz": wz, "wo": wo, "bias": bias,
        })
    return in_maps


def assemble_output(results):
    full = np.empty((B, L, C), np.float32)
    for core in range(N_CORES):
        b, h = divmod(core, 2)
        full[b, h * TOK:(h + 1) * TOK] = results[core]["out"]
    return full.reshape(B, H, W_DIM, C)


def kernel(x, W_in, conv_w, conv_b, W_out):
    nc = _build_nc()
    in_maps = prepare_in_maps(x, W_in, conv_w, conv_b, W_out)
    res = run_bass_kernel_spmd(nc, in_maps, list(range(N_CORES)))
    return assemble_output(res.results)


# revision 22
# speedup vs baseline: 1.1060x; 1.0246x over previous
"""SS2D (stubbed-scan Mamba2D block) Trainium2 kernel.

Math (the four directional scans of the reference collapse to identity):
    xz = x @ W_in^T ; v = causal_depthwise_conv4(xz[:, :512]) + b ; z = xz[:, 512:]
    out = (4 * v * sigmoid(v) * z * sigmoid(z)) @ W_out^T

Sharding: 8 cores, each takes half of one batch image = 8192 tokens
(+3 halo tokens for the causal conv window).

Per-core kernel:
  - x chunk passed as bf16 (8320, 256) = 3 halo + 8192 tokens + 125 zero pad
  - DMA-transpose x into SBUF as xT (2 x (128c, 8320t) bf16)
  - in_proj with conv folded in: v^T tile accumulates 8 matmuls
    (4 taps x 2 c-halves) with tap-shifted rhs column offsets; conv bias
    rides the sigmoid's per-partition bias and a scalar_tensor_tensor op.
  - sigmoid on ScalarE (PSUM -> SBUF bf16), silu products on VectorE
  - out_proj: y^T-stationary matmuls -> out tile (tokens x 256) in PSUM,
    copied to SBUF, contiguous DMA store. The 4x scale is folded into W_out.
"""
import sys

sys.path.insert(0, "/opt/trn_rl_repo")

from contextlib import ExitStack

import numpy as np
import ml_dtypes

import concourse.bass as bass
import concourse.tile as tile
from concourse import bacc, mybir
from concourse.alu_op_type import AluOpType
from concourse.bass_utils import run_bass_kernel_spmd

BF16 = ml_dtypes.bfloat16
N_CORES = 8
B, H, W_DIM, C = 4, 128, 128, 256
L = H * W_DIM
TOK = 8192          # tokens per core
ROWS = 8320         # 3 halo + TOK + 125 pad  (= 65 * 128, mult of 16 for xbar)
T = 512             # token block
NBLK = TOK // T
DI = 512            # d_inner
F32 = mybir.dt.float32
BF16_T = mybir.dt.bfloat16
SIG = mybir.ActivationFunctionType.Sigmoid

_NC_CACHE = None


def _emit_kernel(tc, out_ap, xcs, wx_src, wd_src, wz_t_src, wo_t_src, bias_src):
    nc = tc.nc
    with ExitStack() as ctx:
        consts = ctx.enter_context(tc.tile_pool(name="consts", bufs=1))
        xt_pool = ctx.enter_context(tc.tile_pool(name="xt", bufs=1))
        sb = ctx.enter_context(tc.tile_pool(name="sb", bufs=3))
        ysb = ctx.enter_context(tc.tile_pool(name="ysb", bufs=2))
        osb = ctx.enter_context(tc.tile_pool(name="osb", bufs=16))
        xbsb = ctx.enter_context(tc.tile_pool(name="xbsb", bufs=3))
        pr = ctx.enter_context(tc.tile_pool(name="pr", bufs=2, space="PSUM"))
        pv = ctx.enter_context(tc.tile_pool(name="pv", bufs=2, space="PSUM"))
        pz = ctx.enter_context(tc.tile_pool(name="pz", bufs=1, space="PSUM"))
        po = ctx.enter_context(tc.tile_pool(name="po", bufs=2, space="PSUM"))

        # --- transpose x into SBUF: xT[c][p=c_chan, col=chunk_row] ---
        # (issued before the weight loads so the first blocks' columns land
        # as early as possible; everything stays on the sync ring — mixing
        # DMA-transpose with copies across rings corrupts data on HW)
        xt = [
            xt_pool.tile([128, ROWS], BF16_T, tag=f"xt{c}", name=f"xt{c}")
            for c in range(2)
        ]

        def _load_weights():
            nc.sync.dma_start(out=wx_t, in_=wx_src)
            nc.sync.dma_start(out=wd_t, in_=wd_src)
            nc.sync.dma_start(out=wz_t, in_=wz_t_src)
            nc.sync.dma_start(out=wo_t, in_=wo_t_src)
            nc.sync.dma_start(out=bias_t, in_=bias_src)

        wx_t = consts.tile([128, 8, 128], BF16_T)    # (c, dt) x-branch in_proj
        wd_t = consts.tile([128, 16, 128], BF16_T)   # (k, dt) diag conv taps
        wz_t = consts.tile([128, 8, 128], BF16_T)    # (c, dt) z-branch in_proj
        wo_t = consts.tile([128, 4, 256], BF16_T)    # (dt) out_proj (x4 folded)
        bias_t = consts.tile([128, 4], F32)          # conv bias, column per d-tile
        SEG = 1040
        for s in range(ROWS // SEG):
            for c in range(2):
                nc.sync.dma_start(
                    out=xt[c][:, s * SEG:(s + 1) * SEG],
                    in_=xcs[c][s * SEG:(s + 1) * SEG, :],
                    transpose=True,
                )
            if s == 0:
                _load_weights()

        # --- xb halo tiles for block 0: in_proj of x tokens [-3, 0) ---
        ph = ctx.enter_context(tc.tile_pool(name="ph", bufs=1, space="PSUM"))
        xb_prev = []
        for dt in range(4):
            rh = ph.tile([128, 3], F32, tag="rhalo", name=f"rhalo{dt}")
            for c in range(2):
                nc.tensor.matmul(
                    rh,
                    lhsT=wx_t[:, c * 4 + dt, :],
                    rhs=xt[c][:, 0:3],
                    start=(c == 0), stop=(c == 1),
                )
            hb = consts.tile([128, 3], BF16_T, name=f"xbhalo{dt}")
            nc.scalar.copy(hb, rh)
            xb_prev.append(hb)

        # --- main loop over token blocks ---
        for j in range(NBLK):
            base = T * j
            ys = []
            for dt in range(4):
                bias_ap = bias_t[:, dt:dt + 1]
                # x-branch in_proj (xb covers x tokens [base, base+T));
                # conv taps k<3 additionally read the last 3-k columns of the
                # previous block's xb tile (or the j==0 halo tile).
                r = pr.tile([128, T], F32)
                for c in range(2):
                    nc.tensor.matmul(
                        r,
                        lhsT=wx_t[:, c * 4 + dt, :],
                        rhs=xt[c][:, base + 3:base + 3 + T],
                        start=(c == 0), stop=(c == 1),
                    )
                xb = xbsb.tile([128, T], BF16_T, tag=f"xb{dt}")
                nc.scalar.copy(xb, r)
                prev = xb_prev[dt]
                poff = prev.shape[-1] - 3   # halo tile is (128,3): poff=0
                v = pv.tile([128, T], F32)
                # tap k=3 first: full-width write with start=True clears the
                # bank; remaining taps are pure per-element accumulates.
                nc.tensor.matmul(
                    v,
                    lhsT=wd_t[:, 3 * 4 + dt, :],
                    rhs=xb[:, 0:T],
                    start=True, stop=False,
                    skip_group_check=True,
                )
                for k in range(3):
                    nc.tensor.matmul(
                        v[:, 0:3 - k],
                        lhsT=wd_t[:, k * 4 + dt, :],
                        rhs=prev[:, poff + k:poff + 3],
                        start=False, stop=False,
                        skip_group_check=True,
                    )
                    nc.tensor.matmul(
                        v[:, 3 - k:T],
                        lhsT=wd_t[:, k * 4 + dt, :],
                        rhs=xb[:, 0:T - 3 + k],
                        start=False, stop=(k == 2),
                        skip_group_check=True,
                    )
                xb_prev[dt] = xb
                sv = sb.tile([128, T], BF16_T, tag="sv")
                nc.scalar.activation(sv, v, SIG, bias=bias_ap)

                # z-branch
                z = pz.tile([128, T], F32)
                for c in range(2):
                    nc.tensor.matmul(
                        z,
                        lhsT=wz_t[:, c * 4 + dt, :],
                        rhs=xt[c][:, base + 3:base + 3 + T],
                        start=(c == 0),
                        stop=(c == 1),
                    )
                sz = sb.tile([128, T], BF16_T, tag="sz")
                nc.scalar.activation(sz, z, SIG)

                # silu products: u = (v+b)*sig(v+b), zt = z*sig(z), y = u*zt
                u = sb.tile([128, T], BF16_T, tag="u")
                nc.vector.scalar_tensor_tensor(
                    u, v, bias_ap, sv, AluOpType.add, AluOpType.mult)
                zt = sb.tile([128, T], BF16_T, tag="zt")
                nc.vector.tensor_mul(zt, z, sz)
                y = ysb.tile([128, T], BF16_T, tag=f"y{dt}")
                nc.vector.tensor_mul(y, u, zt)
                ys.append(y)

            # out_proj: y^T tiles are stationary, W_out^T moving.
            # Two 128-token subtiles share one 1-bank PSUM tile (the second
            # group's start=True only clears has_written bits, data persists),
            # halving copies and stores.
            for sp in range(T // 256):
                o_ps = po.tile([128, 512], F32)
                for q in range(2):
                    s = sp * 2 + q
                    for dt in range(4):
                        nc.tensor.matmul(
                            o_ps[:, q * 256:(q + 1) * 256],
                            lhsT=ys[dt][:, s * 128:(s + 1) * 128],
                            rhs=wo_t[:, dt, :],
                            start=(dt == 0),
                            stop=(dt == 3),
                            skip_group_check=True,
                        )
                o_sb = osb.tile([128, 512], F32)
                if sp % 2 == 0:
                    nc.scalar.copy(o_sb, o_ps)
                else:
                    nc.vector.tensor_copy(o_sb, o_ps)
                dram = out_ap[base + sp * 256:base + (sp + 1) * 256, :]
                row = dram.ap[0]
                dram3 = bass.AP(
                    tensor=dram.tensor, offset=dram.offset,
                    ap=[[row[0], 128], [row[0] * 128, 2], dram.ap[1]],
                )
                nc.sync.dma_start(out=dram3, in_=o_sb)


def _build_nc():
    global _NC_CACHE
    if _NC_CACHE is not None:
        return _NC_CACHE
    nc = bacc.Bacc("TRN2", target_bir_lowering=False, debug=False,
                   num_devices=N_CORES)
    xc0 = nc.dram_tensor("xc0", [ROWS, 128], BF16_T, kind="ExternalInput").ap()
    xc1 = nc.dram_tensor("xc1", [ROWS, 128], BF16_T, kind="ExternalInput").ap()
    wx = nc.dram_tensor("wx", [128, 8, 128], BF16_T, kind="ExternalInput").ap()
    wd = nc.dram_tensor("wd", [128, 16, 128], BF16_T, kind="ExternalInput").ap()
    wz = nc.dram_tensor("wz", [128, 8, 128], BF16_T, kind="ExternalInput").ap()
    wo = nc.dram_tensor("wo", [128, 4, 256], BF16_T, kind="ExternalInput").ap()
    bias = nc.dram_tensor("bias", [128, 4], F32, kind="ExternalInput").ap()
    out = nc.dram_tensor("out", [TOK, C], F32, kind="ExternalOutput").ap()

    with tile.TileContext(nc) as tc:
        _emit_kernel(tc, out, (xc0, xc1), wx, wd, wz, wo, bias)
    nc.compile()
    _NC_CACHE = nc
    return nc


def _pack_weights(W_in, conv_w, conv_b, W_out):
    w = conv_w[:, 0, :]                      # (512, 4)
    Wx, Wz = W_in[:DI], W_in[DI:]            # (512, 256) each

    wx = np.empty((128, 8, 128), np.float32)
    for c in range(2):
        for dt in range(4):
            blk = Wx[dt * 128:(dt + 1) * 128, c * 128:(c + 1) * 128]
            wx[:, c * 4 + dt, :] = blk.T
    wd = np.zeros((128, 16, 128), np.float32)
    idx = np.arange(128)
    for k in range(4):
        for dt in range(4):
            wd[idx, k * 4 + dt, idx] = w[dt * 128:(dt + 1) * 128, k]
    wz = np.empty((128, 8, 128), np.float32)
    for c in range(2):
        for dt in range(4):
            blk = Wz[dt * 128:(dt + 1) * 128, c * 128:(c + 1) * 128]
            wz[:, c * 4 + dt, :] = blk.T
    wo = np.empty((128, 4, 256), np.float32)
    WoT4 = 4.0 * W_out.T                     # (512, 256)
    for dt in range(4):
        wo[:, dt, :] = WoT4[dt * 128:(dt + 1) * 128, :]
    bias = conv_b.reshape(4, 128).T.copy()   # (128, 4), column dt
    return (wx.astype(BF16), wd.astype(BF16), wz.astype(BF16), wo.astype(BF16),
            np.ascontiguousarray(bias, np.float32))


def prepare_in_maps(x, W_in, conv_w, conv_b, W_out):
    wx, wd, wz, wo, bias = _pack_weights(
        np.asarray(W_in, np.float32), np.asarray(conv_w, np.float32),
        np.asarray(conv_b, np.float32), np.asarray(W_out, np.float32))

    xf = np.asarray(x, np.float32).reshape(B, L, C)
    in_maps = []
    for core in range(N_CORES):
        b, h = divmod(core, 2)
        chunk = np.zeros((ROWS, C), np.float32)
        if h == 1:
            chunk[0:3] = xf[b, TOK - 3:TOK]
        chunk[3:3 + TOK] = xf[b, h * TOK:(h + 1) * TOK]
        cb = chunk.astype(BF16)
        in_maps.append({
            "xc0": np.ascontiguousarray(cb[:, 0:128]),
            "xc1": np.ascontiguousarray(cb[:, 128:256]),
            "wx": wx, "wd": wd, "wz": wz, "wo": wo, "bias": bias,
        })
    return in_maps


def assemble_output(results):
    full = np.empty((B, L, C), np.float32)
    for core in range(N_CORES):
        b, h = divmod(core, 2)
        full[b, h * TOK:(h + 1) * TOK] = results[core]["out"]
    return full.reshape(B, H, W_DIM, C)


def kernel(x, W_in, conv_w, conv_b, W_out):
    nc = _build_nc()
    in_maps = prepare_in_maps(x, W_in, conv_w, conv_b, W_out)
    res = run_bass_kernel_spmd(nc, in_maps, list(range(N_CORES)))
    return assemble_output(res.results)
